# revision 7
# baseline (speedup 1.0000x reference)
"""BiLSTM-CRF kernel for Trainium2 (8 NeuronCores, SPMD batch-sharded).

Device (Bass/Tile, one launch per call, 8 cores x 4 sequences each):
  phase 0: weights ship sharded 1/8-per-core, AllGather over NeuronLink
           reassembles them on every core (8x less tunnel upload)
  phase 1: xg = [x|1] @ [Wih.T;b] for both directions on PE. x and Wih
           travel as fp16; an fp16 residual matmul restores Wih to ~fp26,
           accumulation in fp32 PSUM (0 tag flips vs full-fp32)
  phase 2: both LSTM recurrences in one position-indexed For_i hardware
           loop (fwd step t uses col t, bwd uses col L-1-t; no data
           reversal anywhere), fp32 throughout
  phase 3: emissions^T = W_out @ hcat on PE, DMA'd out (32KB/core)
Host: embedding gather (shard prep) and Viterbi decode (T=4, ~15ms).

Per call: ~2.2MB/core up, 32KB/core down -- vs ~300MB round trip for the
old gates-on-host split. The axon device init (~80-130s, one-time per
terminal) and first compile are paid at import via warmup().
"""

import sys
import time

for _p in ("/opt/trn_rl_repo", "/root/.axon_site/_ro/trn_rl_repo"):
    if _p not in sys.path:
        sys.path.insert(0, _p)

import numpy as np

try:
    # Persistent XLA compilation cache: run_bass_kernel_spmd re-traces a fresh
    # closure per call, so the in-memory jit cache never hits; the disk cache
    # keyed on HLO bytes does (same nc -> same bytes), saving ~0.2s/call and
    # letting warmup()'s compile serve every later call, across processes.
    import jax as _jax

    _jax.config.update("jax_compilation_cache_dir", "/root/.jax_comp_cache")
    _jax.config.update("jax_persistent_cache_min_compile_time_secs", 0.0)
    _jax.config.update("jax_persistent_cache_min_entry_size_bytes", 0)
except Exception:
    pass

B, L, V, E, H, T = 32, 512, 100000, 300, 256, 4
NCORES = 8
S = B // NCORES          # sequences per core
E1 = 384                 # E rows + bias row + pad to 3*128 (SBUF layout)
E2 = 301                 # rows actually uploaded (E + bias row)
KE = E1 // 128           # 3 contraction blocks for the input projection
G4 = 4 * H               # 1024 gates per direction
MB = G4 // 128           # 8 gate M-blocks per direction
KH = H // 128            # 2 contraction blocks for the recurrence
KC = 2 * H // 128        # 4 contraction blocks for the emission projection

LAST_DEVICE_NS = None    # wall-time of the device execution, for test.py
_NC_CACHE = {}


def _bilstm_ir(tc, L_steps, xT, wihT, wihTr, whhT, woutT, emisT):
    """Emit the full BiLSTM IR. All args are 2D DRAM APs (external params
    or AllGather outputs):
    xT    [E2, S*L]   tokens col = s*L + t; row 300 = 1.0 (the bias input)
    wihT  [E2, 2*G4]  f16 hi part; cols 0:G4 fwd, G4: bwd; gate order i,f,o,g
    wihTr [E2, 2*G4]  f16 residual of (f32 wih - f16 wihT)
    whhT  [128, 4*G4] (k, m): k in {f0,f1,b0,b1} 128-blocks of h; m gate dim
    woutT [128, KC*T] (k, t): k over hcat = [h_f | h_b] 128-blocks
    emisT [T, S*L]    output emissions (pre b_out), col = s*L + t
    """
    import concourse.mybir as mybir
    from concourse.bass import ds

    from contextlib import ExitStack

    nc = tc.nc
    NT = S * L_steps
    f32 = mybir.dt.float32
    f16 = mybir.dt.float16
    dt_p1 = f16   # xT, wihT (phase-1 matmul operands)
    dt_rec = f32  # whh, hseq, wout (recurrence + emission operands)
    ACT = mybir.ActivationFunctionType

    ctx = ExitStack()
    pool = ctx.enter_context(tc.tile_pool(name="main", bufs=1))

    # --- load inputs to SBUF (E-dim tensors ship 301 rows; tail zeroed) ---
    xT_sb = pool.tile([128, KE, NT], dt_p1, tag="slotA")
    wihT_sb = pool.tile([128, KE, 2 * G4], dt_p1, tag="slotB")
    wihTr_sb = pool.tile([128, KE, 2 * G4], dt_p1, tag="wihr")
    whh_sb = pool.tile([128, 2 * KH, G4], dt_rec, tag="whh")
    wout_sb = pool.tile([128, KC, T], dt_rec, tag="wout")
    PT = E2 - 256  # partial-block rows (45)
    for sb, dram in ((xT_sb, xT), (wihT_sb, wihT), (wihTr_sb, wihTr)):
        nc.vector.memset(sb[:, KE - 1, :], 0.0)  # zero last k-block; DMA then fills rows 0:45
        nc.sync.dma_start(
            sb[:, : KE - 1, :], dram[:256].rearrange("(k p) n -> p k n", p=128)
        )
        nc.sync.dma_start(sb[:PT, KE - 1, :], dram[256:E2])
    nc.sync.dma_start(whh_sb[:], whhT.rearrange("p (k m) -> p k m", m=G4))
    nc.sync.dma_start(wout_sb[:], woutT.rearrange("p (k t) -> p k t", t=T))

    xg = [
        pool.tile([128, MB, NT], f32, tag="xg_f", name="xg_f"),
        pool.tile([128, MB, NT], f32, tag="xg_b", name="xg_b"),
    ]

    # --- phase 1: input projections, out = wihT.T @ xT (per 128-col M block) ---
    NCHUNK = 512
    with tc.tile_pool(name="ps1", bufs=4, space="PSUM") as ps1:
        for d in range(2):              # direction
            for m in range(MB):         # gate M-block
                for c0 in range(0, NT, NCHUNK):
                    cw = min(NCHUNK, NT - c0)
                    pt = ps1.tile([128, NCHUNK], f32, tag="p1", name="p1")
                    for w_i, w_sb in enumerate((wihT_sb, wihTr_sb)):
                        for k in range(KE):
                            nc.tensor.matmul(
                                pt[:, :cw],
                                w_sb[:, k, d * G4 + m * 128 : d * G4 + (m + 1) * 128],
                                xT_sb[:, k, c0 : c0 + cw],
                                start=(w_i == 0 and k == 0),
                                stop=(w_i == 1 and k == KE - 1),
                            )
                    nc.vector.tensor_copy(out=xg[d][:, m, c0 : c0 + cw], in_=pt[:, :cw])

    # --- phase 2: the two recurrences ---
    # hseq layout [128, KH, NT]; gate/act working layout [128, m, s].
    hseq = [
        pool.tile([128, KH, NT], dt_rec, tag="slotA", name="hseq_f"),
        pool.tile([128, KH, NT], dt_rec, tag="slotB", name="hseq_b"),
    ]
    acts = [pool.tile([128, MB, S], f32, tag=f"acts{d}", name=f"acts{d}") for d in range(2)]
    gsum = [pool.tile([128, MB, S], f32, tag=f"gsum{d}", name=f"gsum{d}") for d in range(2)]
    cc = [pool.tile([128, KH, S], f32, tag=f"c{d}", name=f"c{d}") for d in range(2)]
    tmp = [pool.tile([128, KH, S], f32, tag=f"tmp{d}", name=f"tmp{d}") for d in range(2)]
    tch = [pool.tile([128, KH, S], f32, tag=f"tch{d}", name=f"tch{d}") for d in range(2)]

    xg_r = [t.rearrange("p m (s t) -> p m s t", s=S) for t in xg]
    hseq_r = [t.rearrange("p k (s t) -> p k s t", s=S) for t in hseq]

    def lstm_tail(d, gate_src):
        """Apply gate activations; returns acts[d] laid out [128, m, s]."""
        a = acts[d]
        nc.scalar.activation(a[:, 0:6, :], gate_src[:, 0:6, :], ACT.Sigmoid)
        nc.scalar.activation(a[:, 6:8, :], gate_src[:, 6:8, :], ACT.Tanh)
        return a

    def lstm_step0(d, col):
        a = lstm_tail(d, xg_r[d][:, :, :, col])
        # c0 = sig(i) * tanh(g);  h0 = sig(o) * tanh(c0)
        nc.vector.tensor_mul(out=cc[d][:], in0=a[:, 0:2, :], in1=a[:, 6:8, :])
        nc.scalar.activation(tch[d][:], cc[d][:], ACT.Tanh)
        nc.vector.tensor_mul(
            out=hseq_r[d][:, :, :, col], in0=a[:, 4:6, :], in1=tch[d][:]
        )

    def lstm_step(ps2, d, col_r, col_g, col_w):
        pt = ps2.tile([128, MB, S], f32, tag=f"p2_{d}", name=f"p2_{d}")
        for m in range(MB):
            for k in range(KH):
                nc.tensor.matmul(
                    pt[:, m, :],
                    whh_sb[:, d * KH + k, m * 128 : (m + 1) * 128],
                    hseq_r[d][:, k, :, col_r],
                    start=(k == 0),
                    stop=(k == KH - 1),
                )
        nc.vector.tensor_add(out=gsum[d][:], in0=pt[:], in1=xg_r[d][:, :, :, col_g])
        a = lstm_tail(d, gsum[d])
        # c = sig(f)*c + sig(i)*tanh(g);  h = sig(o)*tanh(c)
        nc.vector.tensor_mul(out=tmp[d][:], in0=a[:, 0:2, :], in1=a[:, 6:8, :])
        nc.vector.tensor_mul(out=cc[d][:], in0=a[:, 2:4, :], in1=cc[d][:])
        nc.vector.tensor_add(out=cc[d][:], in0=cc[d][:], in1=tmp[d][:])
        nc.scalar.activation(tch[d][:], cc[d][:], ACT.Tanh)
        nc.vector.tensor_mul(
            out=hseq_r[d][:, :, :, col_w], in0=a[:, 4:6, :], in1=tch[d][:]
        )

    lstm_step0(0, 0)
    lstm_step0(1, L_steps - 1)
    with tc.tile_pool(name="ps2", bufs=2, space="PSUM") as ps2:
        with tc.For_i(0, L_steps - 1, 1) as i:
            lstm_step(ps2, 0, ds(i, 1), ds(i + 1, 1), ds(i + 1, 1))
            lstm_step(
                ps2,
                1,
                ds(L_steps - 1 - i, 1),
                ds(L_steps - 2 - i, 1),
                ds(L_steps - 2 - i, 1),
            )

    # --- phase 3: emissions^T = woutT.T @ hcatT ---
    emis_sb = pool.tile([T, NT], f32, tag="emis")
    with tc.tile_pool(name="ps3", bufs=2, space="PSUM") as ps3:
        for c0 in range(0, NT, NCHUNK):
            cw = min(NCHUNK, NT - c0)
            pt = ps3.tile([T, NCHUNK], f32, tag="p3", name="p3")
            for k in range(KC):
                nc.tensor.matmul(
                    pt[:, :cw],
                    wout_sb[:, k, :],
                    hseq[k // KH][:, k % KH, c0 : c0 + cw],
                    start=(k == 0),
                    stop=(k == KC - 1),
                )
            nc.vector.tensor_copy(out=emis_sb[:, c0 : c0 + cw], in_=pt[:, :cw])
    nc.sync.dma_start(emisT, emis_sb[:])
    ctx.close()


def build_nc(L_steps=L, use_cc=True):
    import concourse.bacc as bacc
    import concourse.mybir as mybir
    from concourse.tile import TileContext

    NT = S * L_steps
    f32 = mybir.dt.float32
    f16 = mybir.dt.float16
    nc = bacc.Bacc(num_devices=NCORES if use_cc else None)
    xT = nc.declare_dram_parameter("xT", [E2, NT], f16, isOutput=False)
    emisT = nc.declare_dram_parameter("emisT", [T, NT], f32, isOutput=True)
    n16 = 2 * E2 * 2 * G4
    n32 = 128 * 2 * KH * G4 + 128 * KC * T
    with TileContext(nc) as tc:
        if use_cc:
            # each core ships 1/8 of the weight bytes; AllGather reassembles
            wsh16 = nc.declare_dram_parameter("wsh16", [n16 // NCORES], f16, isOutput=False)
            wsh32 = nc.declare_dram_parameter("wsh32", [n32 // NCORES], f32, isOutput=False)
            b16 = nc.dram_tensor("b16", [n16 // NCORES], f16)
            b32 = nc.dram_tensor("b32", [n32 // NCORES], f32)
            g16 = nc.dram_tensor("g16", [n16], f16, addr_space="Shared")
            g32 = nc.dram_tensor("g32", [n32], f32, addr_space="Shared")
            nc.sync.dma_start(b16[:], wsh16[:])
            nc.sync.dma_start(b32[:], wsh32[:])
            groups = [list(range(NCORES))]
            nc.gpsimd.collective_compute(
                "AllGather", mybir.AluOpType.bypass, groups, [b16[:]], [g16[:]]
            )
            nc.gpsimd.collective_compute(
                "AllGather", mybir.AluOpType.bypass, groups, [b32[:]], [g32[:]]
            )
            f16a = g16[:]
            f32a = g32[:]
            nwih = E2 * 2 * G4
            wihT = f16a[0:nwih].rearrange("(e n) -> e n", n=2 * G4)
            wihTr = f16a[nwih : 2 * nwih].rearrange("(e n) -> e n", n=2 * G4)
            nwhh = 128 * 2 * KH * G4
            whhT = f32a[0:nwhh].rearrange("(p n) -> p n", n=2 * KH * G4)
            woutT = f32a[nwhh:].rearrange("(p n) -> p n", n=KC * T)
        else:
            wihT = nc.declare_dram_parameter("wihT", [E2, 2 * G4], f16, isOutput=False)[:]
            wihTr = nc.declare_dram_parameter("wihTr", [E2, 2 * G4], f16, isOutput=False)[:]
            whhT = nc.declare_dram_parameter("whhT", [128, 2 * KH * G4], f32, isOutput=False)[:]
            woutT = nc.declare_dram_parameter("woutT", [128, KC * T], f32, isOutput=False)[:]
        _bilstm_ir(tc, L_steps, xT[:], wihT, wihTr, whhT, woutT, emisT[:])
    nc.finalize()
    return nc


_PERM = None


def _gate_perm():
    """Row permutation taking PyTorch gate order (i,f,g,o) to (i,f,o,g)."""
    global _PERM
    if _PERM is None:
        r = np.arange(G4)
        _PERM = np.concatenate([r[0:256], r[256:512], r[768:1024], r[512:768]])
    return _PERM


def host_inputs(x, Wih_f, b_f, Wih_b, b_b, Whh_f, Whh_b, W_out, L_steps=L, use_cc=True):
    """Build per-core input maps. x: [B, L, E] fp32 (B divisible by NCORES)."""
    NT = S * L_steps

    # gate reorder (i,f,g,o) -> (i,f,o,g) as three contiguous slice copies
    def perm_rows_T(dst, W2d):
        dst[:, 0 : 2 * H] = W2d[0 : 2 * H].T
        dst[:, 2 * H : 3 * H] = W2d[3 * H : 4 * H].T
        dst[:, 3 * H : 4 * H] = W2d[2 * H : 3 * H].T

    def perm_vec(v):
        return np.concatenate([v[0 : 2 * H], v[3 * H : 4 * H], v[2 * H : 3 * H]])

    wih32 = np.zeros((E2, 2 * G4), np.float32)
    perm_rows_T(wih32[:E, 0:G4], Wih_f)
    wih32[E, 0:G4] = perm_vec(b_f)
    perm_rows_T(wih32[:E, G4:], Wih_b)
    wih32[E, G4:] = perm_vec(b_b)
    wihT = wih32.astype(np.float16)
    wihTr = (wih32 - wihT.astype(np.float32)).astype(np.float16)

    def whh_pack(Whh):
        # [128, KH, G4]: (part, k) = h-dim, m = gate dim (reordered)
        WT = np.empty((H, G4), np.float32)
        perm_rows_T(WT, Whh)
        return WT.reshape(KH, 128, G4).transpose(1, 0, 2).reshape(128, KH * G4)

    whhT = np.concatenate([whh_pack(Whh_f), whh_pack(Whh_b)], axis=1)
    woutT = np.ascontiguousarray(
        W_out.T.reshape(KC, 128, T).transpose(1, 0, 2)
    ).reshape(128, KC * T)

    if use_cc:
        flat16 = np.concatenate([wihT.ravel(), wihTr.ravel()])
        flat32 = np.concatenate([whhT.ravel(), woutT.ravel()])
        s16 = flat16.size // NCORES
        s32 = flat32.size // NCORES

    xTall = np.empty((NCORES, E2, NT), np.float16)
    xTall[:, :E] = x.reshape(NCORES, NT, E).transpose(0, 2, 1)
    xTall[:, E] = 1.0

    in_maps = []
    for c in range(NCORES):
        xTp = xTall[c]
        if use_cc:
            in_maps.append(
                {
                    "xT": xTp,
                    "wsh16": flat16[c * s16 : (c + 1) * s16],
                    "wsh32": flat32[c * s32 : (c + 1) * s32],
                }
            )
        else:
            in_maps.append(
                {"xT": xTp, "wihT": wihT, "wihTr": wihTr, "whhT": whhT, "woutT": woutT}
            )
    return in_maps


def _viterbi(emissions, mask, transitions, start_trans, end_trans):
    Bn, Ln, _ = emissions.shape
    m = mask.astype(bool)
    score = start_trans + emissions[:, 0]
    history = np.empty((Ln - 1, Bn, T), np.int32)
    for t in range(1, Ln):
        cand = score[:, :, None] + transitions[None] + emissions[:, t][:, None, :]
        history[t - 1] = np.argmax(cand, axis=1).astype(np.int32)
        new = np.max(cand, axis=1)
        score = np.where(m[:, t][:, None], new, score)
    score = score + end_trans
    tag = np.argmax(score, axis=-1).astype(np.int32)
    tags = np.empty((Bn, Ln), np.int32)
    tags[:, Ln - 1] = tag
    rows = np.arange(Bn)
    for t in range(Ln - 2, -1, -1):
        prev = history[t][rows, tag]
        tag = np.where(m[:, t + 1], prev, tag).astype(np.int32)
        tags[:, t] = tag
    return tags * mask.astype(np.int32)


def _get_nc(use_cc=True):
    key = "nc_cc" if use_cc else "nc"
    if key not in _NC_CACHE:
        _NC_CACHE[key] = build_nc(use_cc=use_cc)
    return _NC_CACHE[key]


def _run_device_once(in_maps, use_cc):
    global LAST_DEVICE_NS
    from concourse.bass_utils import run_bass_kernel_spmd

    nc = _get_nc(use_cc)
    t0 = time.perf_counter()
    res = run_bass_kernel_spmd(nc, in_maps, list(range(NCORES)))
    LAST_DEVICE_NS = int((time.perf_counter() - t0) * 1e9)
    if getattr(res, "exec_time_ns", None):
        LAST_DEVICE_NS = int(res.exec_time_ns)
    return [np.asarray(r["emisT"]) for r in res.results]


def _run_device(x, wargs):
    if not _NC_CACHE.get("cc_broken"):
        try:
            return _run_device_once(host_inputs(x, *wargs, use_cc=True), use_cc=True)
        except Exception:
            _NC_CACHE["cc_broken"] = True
    return _run_device_once(host_inputs(x, *wargs, use_cc=False), use_cc=False)


def kernel(
    word_ids,
    mask,
    label_ids,
    emb,
    Wih_f,
    Whh_f,
    b_f,
    Wih_b,
    Whh_b,
    b_b,
    W_out,
    b_out,
    transitions,
    start_trans,
    end_trans,
):
    word_ids = np.asarray(word_ids, np.int32)
    mask = np.asarray(mask, np.int32)
    emb = np.asarray(emb, np.float32)

    x = emb[word_ids]  # [B, L, E] embedding gather (host; shard prep)

    wargs = (
        np.asarray(Wih_f, np.float32),
        np.asarray(b_f, np.float32),
        np.asarray(Wih_b, np.float32),
        np.asarray(b_b, np.float32),
        np.asarray(Whh_f, np.float32),
        np.asarray(Whh_b, np.float32),
        np.asarray(W_out, np.float32),
    )
    outs = _run_device(x, wargs)

    # emisT [T, S*L] per core -> emissions [B, L, T]
    emissions = np.concatenate(
        [o.reshape(T, S, L).transpose(1, 2, 0) for o in outs], axis=0
    ) + np.asarray(b_out, np.float32)

    return _viterbi(
        emissions,
        mask,
        np.asarray(transitions, np.float32),
        np.asarray(start_trans, np.float32),
        np.asarray(end_trans, np.float32),
    ).astype(np.int32)


def warmup():
    """Pay axon device init + one compile at import/module-load time."""
    try:
        zero = np.zeros((B, L, E), np.float32)
        zw = np.zeros((G4, E), np.float32)
        zb = np.zeros((G4,), np.float32)
        zh = np.zeros((G4, H), np.float32)
        zo = np.zeros((T, 2 * H), np.float32)
        _run_device(zero, (zw, zb, zw, zb, zh, zh, zo))
    except Exception:
        pass


import os as _os

if not _os.environ.get("BILSTM_KERNEL_NO_WARMUP"):
    warmup()
